# revision 20
# baseline (speedup 1.0000x reference)
# ListFold loss (exponential transform, beta=1) on 8 Trainium2 NeuronCores.
#
# Math: with sp = pred sorted by target descending, the reference computes
#   loss = sum_i log(den_i) - (sp[i] - sp[n-1-i]),  i in [0, n/2)
#   den_i = (cp[n-i]-cp[i]) * (cm[n-i]-cm[i]) - (n-2i)
# where cp/cm are prefix sums of exp(+-sp). Re-indexing from the middle
# outward with t = n/2-1-i, u[t] = sp[n/2-1-t], v[t] = sp[n/2+t]:
#   s_plus(t)  = cumsum_incl(exp(u)+exp(v))[t]      (= cp[n-i]-cp[i])
#   s_minus(t) = cumsum_incl(exp(-u)+exp(-v))[t]
#   loss = sum_t log(s_plus*s_minus - (2t+2)) - (u[t]-v[t])
# Window sums are exact (no differencing of large prefix sums) and, by
# Cauchy-Schwarz, s_plus*s_minus >= L^2, so den >= L(L-1) > 0: the bf16
# element streams below cannot produce a negative log argument. The
# log_num part enters only through a global sum: sum_t (u-v) is computed
# on the host during unshard (two block sums, no device work).
#
# Sharding: the pair index t is split into 8 contiguous blocks, one per
# core, laid out [128 partitions x 4096] partition-major. Each core scans
# its block chunk-by-chunk along the free axis (tensor_tensor_scan chained
# through the previous chunk's last column), resolves the partition-axis
# carry with a strict-triangular matmul, and the cross-core carry with an
# [8,10] AllGather of per-(core,chunk) totals (scan-style carry
# exchange). Chunk totals ride accum_out on the wp/wm pair-sum STTs; the
# partition/chunk folding happens inside the PE matmuls (hot path:
# ones^T @ aw for the trigger, strict @ aw + mask @ allgather for the
# carry), so the collective trigger only waits on the last pair-sum - on
# the slowest-starting core every cycle of trigger delay lands directly
# on the collective's completion time. Edge chunks are half-width so the
# first exp starts sooner and the last pair-sum ends sooner. Elementwise
# streams are bf16 (2x DVE tensor_tensor; scan state is fp32 internally
# per the ISA). Phase B applies the carry on the otherwise-idle PE as two
# diag(carry) matmuls accumulated in PSUM, leaving one DVE add + one Ln
# per chunk after the collective. Per-core partial losses are summed on
# the host (the unshard step). The argsort is int bookkeeping done on the
# host while sharding (XLA cannot sort on trn2 at all).
#
# (A direct peer remote_dma_broadcast exchange instead of the collective
# was probe-verified functional but takes 6-8 ms under this runtime's
# host-emulated SWDGE path, so the hardware collective stays.)

import numpy as np

N = 8388608
H = N // 2          # pairs
NCORES = 8
B = H // NCORES     # pairs per core
P = 128
C = B // P          # 4096 free-dim columns
FLIST = [256, 1024, 1024, 1024, 768]   # phase chunk widths
NCH = len(FLIST)
OFFS = [sum(FLIST[:i]) for i in range(NCH)]
assert sum(FLIST) == C
KW = 2 * NCH        # aw16 width: wp cols then wm cols

_CACHE = {}


def _build_nc():
    import concourse.bacc as bacc
    import concourse.mybir as mybir
    import concourse.tile as tile

    dt = mybir.dt
    f32 = dt.float32
    bf16 = dt.bfloat16
    Alu = mybir.AluOpType
    Act = mybir.ActivationFunctionType

    nc = bacc.Bacc("TRN2", target_bir_lowering=False, debug=False,
                   num_devices=NCORES)

    u_in = nc.dram_tensor("u_in", [P, C], f32, kind="ExternalInput").ap()
    v_in = nc.dram_tensor("v_in", [P, C], f32, kind="ExternalInput").ap()
    maskbc = nc.dram_tensor("maskbc", [NCORES, P], f32, kind="ExternalInput").ap()
    strict = nc.dram_tensor("strict", [P, P], f32, kind="ExternalInput").ap()
    ident = nc.dram_tensor("ident", [P, P], bf16, kind="ExternalInput").ap()
    ones_col = nc.dram_tensor("ones_col", [P, 1], f32, kind="ExternalInput").ap()
    neg_lbase = nc.dram_tensor("neg_lbase", [P, 1], f32, kind="ExternalInput").ap()
    out_part = nc.dram_tensor("partial", [1, 1], f32, kind="ExternalOutput").ap()

    with tile.TileContext(nc) as tc:
        with (
            tc.tile_pool(name="const", bufs=1) as constp,
            tc.tile_pool(name="big", bufs=1) as bigp,
            tc.tile_pool(name="work", bufs=2) as workp,
            tc.tile_pool(name="small", bufs=1) as smallp,
            tc.tile_pool(name="acc", bufs=1) as accp,
            tc.tile_pool(name="psum", bufs=1, space="PSUM") as psump,
            tc.tile_pool(name="psumB", bufs=1, space="PSUM") as psumbp,
            tc.tile_pool(name="dram", bufs=1, space="DRAM") as dramp,
        ):
            strict_t = constp.tile([P, P], f32, tag="strict")
            ident_t = constp.tile([P, P], bf16, tag="ident")
            maskbc_t = constp.tile([NCORES, P], f32, tag="maskbc")
            ones_col_t = constp.tile([P, 1], f32, tag="ones_col")
            neg_lbase_t = constp.tile([P, 1], f32, tag="neg_lbase")

            # L(t_local) = 2*(p*C + c) + 2; bf16 rounding of L is harmless:
            # den >= L(L-1) makes the relative den error <= 2^-9 * L/(L-1).
            iota_t = bigp.tile([P, C], bf16, tag="iota")
            nc.gpsimd.iota(iota_t[:], pattern=[[2, C]], base=2,
                           channel_multiplier=2 * C,
                           allow_small_or_imprecise_dtypes=True)

            wp_t = bigp.tile([P, C], bf16, tag="wp")   # exp(u)+exp(v)
            wm_t = bigp.tile([P, C], bf16, tag="wm")   # exp(-u)+exp(-v)
            sp_t = bigp.tile([P, C], bf16, tag="sp")   # running scan of wp
            sm_t = bigp.tile([P, C], bf16, tag="sm")   # running scan of wm
            x1_t = bigp.tile([P, C], bf16, tag="x1")   # sp*sm - iota

            aw = accp.tile([P, KW], f32, tag="aw")     # chunk row totals
            aln = accp.tile([P, NCH], f32, tag="aln")  # chunk row sums of ln

            # ---- phase A pass 1: exps + pair sums (with accum -> chunk
            # totals). Emitted before any scan so the trigger-critical
            # STTs sit at the front of the DVE queue; the scans have a
            # whole collective window of slack. ----
            for c in range(NCH):
                F = FLIST[c]
                o = OFFS[c]
                cs = slice(o, o + F)
                u_t = workp.tile([P, F], f32, tag=f"u{F}")
                v_t = workp.tile([P, F], f32, tag=f"v{F}")
                nc.sync.dma_start(u_t[:], u_in[:, cs])
                nc.sync.dma_start(v_t[:], v_in[:, cs])

                eu = workp.tile([P, F], bf16, tag=f"eu{F}")
                ev = workp.tile([P, F], bf16, tag=f"ev{F}")
                emu = workp.tile([P, F], bf16, tag=f"emu{F}")
                emv = workp.tile([P, F], bf16, tag=f"emv{F}")
                nc.scalar.activation(eu[:], u_t[:], Act.Exp)
                nc.scalar.activation(ev[:], v_t[:], Act.Exp)
                nc.scalar.activation(emu[:], u_t[:], Act.Exp, scale=-1.0)
                nc.scalar.activation(emv[:], v_t[:], Act.Exp, scale=-1.0)

                nc.vector.scalar_tensor_tensor(
                    out=wp_t[:, cs], in0=eu[:], scalar=0.0, in1=ev[:],
                    op0=Alu.add, op1=Alu.add, accum_out=aw[:, c:c + 1])
                nc.vector.scalar_tensor_tensor(
                    out=wm_t[:, cs], in0=emu[:], scalar=0.0, in1=emv[:],
                    op0=Alu.add, op1=Alu.add,
                    accum_out=aw[:, NCH + c:NCH + c + 1])

            # ---- phase A pass 2: chained chunk scans + carry-independent
            # x1 (fills the collective window) ----
            for c in range(NCH):
                F = FLIST[c]
                o = OFFS[c]
                cs = slice(o, o + F)
                ip = 0.0 if c == 0 else sp_t[:, o - 1:o]
                im = 0.0 if c == 0 else sm_t[:, o - 1:o]
                # data1 is ignored (op1=bypass) but points at the LAST
                # chunk's wm window: a deliberate dependency on the final
                # pair-sum STT so no scan can slip ahead of the
                # trigger-critical STT chain in the DVE queue.
                tail_dep_p = wm_t[:, C - F:C]
                tail_dep_m = wm_t[:, C - F:C]
                nc.vector.tensor_tensor_scan(
                    sp_t[:, cs], wp_t[:, cs], tail_dep_p, ip,
                    Alu.add, Alu.bypass)
                nc.vector.tensor_tensor_scan(
                    sm_t[:, cs], wm_t[:, cs], tail_dep_m, im,
                    Alu.add, Alu.bypass)

                prod = workp.tile([P, F], bf16, tag=f"prod{F}")
                nc.vector.tensor_mul(prod[:], sp_t[:, cs], sm_t[:, cs])
                nc.vector.tensor_sub(x1_t[:, cs], prod[:], iota_t[:, cs])

            # consts are needed from the carry stage on - issue their DMAs
            # after the phase-A loads so chunk 0 starts sooner
            nc.sync.dma_start(strict_t[:], strict)
            nc.sync.dma_start(ident_t[:], ident)
            nc.sync.dma_start(maskbc_t[:], maskbc)
            nc.sync.dma_start(ones_col_t[:], ones_col)
            nc.sync.dma_start(neg_lbase_t[:], neg_lbase)

            # ---- trigger path: my totals row = ones^T @ aw, then gather ----
            contrib_ps = psump.tile([1, KW], f32, tag="contrib")
            nc.tensor.matmul(contrib_ps[:], ones_col_t[:], aw[:],
                             start=True, stop=True)
            contrib_sb = smallp.tile([1, KW], f32, tag="contrib_sb")
            nc.scalar.copy(contrib_sb[:], contrib_ps[:])

            cc_in = dramp.tile([1, KW], f32, tag="cc_in")
            cc_out = dramp.tile([NCORES, KW], f32, tag="cc_out")
            nc.sync.dma_start(cc_in[:], contrib_sb[:])
            nc.gpsimd.collective_compute(
                "AllGather", Alu.bypass,
                replica_groups=[list(range(NCORES))],
                ins=[cc_in.opt()], outs=[cc_out.opt()])
            allt = smallp.tile([NCORES, KW], f32, tag="allt")
            nc.sync.dma_start(allt[:], cc_out[:])

            # carry = strict-local partition prefix + earlier cores' totals,
            # both PSUM-accumulated, then folded over chunks by two reduces
            carry_ps = psump.tile([P, KW], f32, tag="carry")
            nc.tensor.matmul(carry_ps[:], strict_t[:], aw[:],
                             start=True, stop=False)
            nc.tensor.matmul(carry_ps[:], maskbc_t[:], allt[:],
                             start=False, stop=True)
            carry_sb = smallp.tile([P, 2], f32, tag="carry_sb")
            nc.vector.tensor_reduce(carry_sb[:, 0:1], carry_ps[:, 0:NCH],
                                    axis=mybir.AxisListType.X, op=Alu.add)
            nc.vector.tensor_reduce(carry_sb[:, 1:2], carry_ps[:, NCH:KW],
                                    axis=mybir.AxisListType.X, op=Alu.add)

            # warm the Ln table while ACT is idle (input: any ready f32)
            lnwarm = smallp.tile([P, 1], f32, tag="lnwarm")
            nc.scalar.activation(lnwarm[:], aw[:, 0:1], Act.Ln)

            # diag(Cp), diag(Cm) as bf16 lhsT for the phase-B PE matmuls
            dcp = smallp.tile([P, P], bf16, tag="dcp")
            dcm = smallp.tile([P, P], bf16, tag="dcm")
            nc.vector.tensor_scalar(
                out=dcp[:], in0=ident_t[:], scalar1=carry_sb[:, 0:1],
                scalar2=None, op0=Alu.mult)
            nc.vector.tensor_scalar(
                out=dcm[:], in0=ident_t[:], scalar1=carry_sb[:, 1:2],
                scalar2=None, op0=Alu.mult)

            # bias = Cp*Cm - 2kB (per-partition scalar for the Ln)
            cpcm = smallp.tile([P, 1], f32, tag="cpcm")
            nc.vector.tensor_scalar(
                out=cpcm[:], in0=carry_sb[:, 0:1], scalar1=carry_sb[:, 1:2],
                scalar2=None, op0=Alu.mult)
            bias_t = smallp.tile([P, 1], f32, tag="bias_t")
            nc.vector.tensor_add(bias_t[:], cpcm[:], neg_lbase_t[:])

            # ---- phase B: den = x1 + Cp*sm + Cm*sp + (CpCm - 2kB), log.
            # Carry products on the PE: sub-tiles grouped in fours so each
            # diag weight-load covers four matmuls; DVE adds x1 and folds
            # each chunk's ln row-sums (no ACT accumulator reads); the
            # per-chunk partial accumulates in PSUM right behind its Ln. ----
            HB = 512
            subs = []  # (chunk, lo, width)
            for c in range(NCH):
                o, F = OFFS[c], FLIST[c]
                lo = o
                while lo < o + F:
                    w = min(HB, o + F - lo)
                    subs.append((c, lo, w))
                    lo += w
            t2s = {c: workp.tile([P, FLIST[c]], bf16, tag=f"t2_{c}",
                                  name=f"t2_{c}")
                   for c in range(NCH)}
            part_ps = psump.tile([1, 1], f32, tag="part")
            done_subs = {c: 0 for c in range(NCH)}
            nsub = {c: 0 for c in range(NCH)}
            for c, lo, w in subs:
                nsub[c] += 1
            for g in range(0, len(subs), 4):
                grp = subs[g:g + 4]
                pss = []
                for i, (c, lo, w) in enumerate(grp):
                    ps_h = psumbp.tile([P, HB], f32, tag=f"psB{i}",
                                       name=f"psB{i}_{g}")
                    nc.tensor.matmul(ps_h[:, 0:w], dcp[:], sm_t[:, lo:lo + w],
                                     start=True, stop=False)
                    pss.append(ps_h)
                for i, (c, lo, w) in enumerate(grp):
                    nc.tensor.matmul(pss[i][:, 0:w], dcm[:],
                                     sp_t[:, lo:lo + w],
                                     start=False, stop=True)
                for i, (c, lo, w) in enumerate(grp):
                    o = OFFS[c]
                    nc.vector.tensor_add(t2s[c][:, lo - o:lo - o + w],
                                         x1_t[:, lo:lo + w], pss[i][:, 0:w])
                    done_subs[c] += 1
                    if done_subs[c] == nsub[c]:
                        ln_o = workp.tile([P, FLIST[c]], bf16, tag=f"ln_{c}",
                                          name=f"lnout_{c}")
                        nc.scalar.activation(ln_o[:], t2s[c][:], Act.Ln,
                                             bias=bias_t[:])
                        nc.vector.tensor_reduce(
                            aln[:, c:c + 1], ln_o[:],
                            axis=mybir.AxisListType.X, op=Alu.add)
                        nc.tensor.matmul(part_ps[:], ones_col_t[:],
                                         aln[:, c:c + 1],
                                         start=(c == 0), stop=(c == NCH - 1))

            part_sb = smallp.tile([1, 1], f32, tag="part_sb")
            nc.scalar.copy(part_sb[:], part_ps[:])
            nc.sync.dma_start(out_part, part_sb[:])

    nc.compile()
    return nc


def _get_nc():
    if "nc" not in _CACHE:
        _CACHE["nc"] = _build_nc()
    return _CACHE["nc"]


def _make_in_maps(pred, target):
    pred = np.ascontiguousarray(np.asarray(pred, dtype=np.float32))
    target = np.ascontiguousarray(np.asarray(target, dtype=np.float32))
    assert pred.shape == (N,) and target.shape == (N,)

    order = np.argsort(-target, kind="stable")  # matches jnp stable argsort
    sp = pred[order]
    u = sp[H - 1:: -1]  # sp[H-1-t]
    v = sp[H:]          # sp[H+t]

    strict = np.triu(np.ones((P, P), np.float32), 1)  # [q,p]=1 iff q<p
    from ml_dtypes import bfloat16 as _bf
    ident = np.eye(P, dtype=np.float32).astype(_bf)  # 0/1: exact in bf16
    ones_col = np.ones((P, 1), np.float32)

    in_maps = []
    for k in range(NCORES):
        mask = np.zeros((NCORES, P), np.float32)
        mask[:k, :] = 1.0
        in_maps.append({
            "u_in": np.ascontiguousarray(u[k * B:(k + 1) * B].reshape(P, C)),
            "v_in": np.ascontiguousarray(v[k * B:(k + 1) * B].reshape(P, C)),
            "maskbc": mask,
            "strict": strict,
            "ident": ident,
            "ones_col": ones_col,
            "neg_lbase": np.full((P, 1), -2.0 * k * B, np.float32),
        })
    return in_maps, u, v


def _run(in_maps, trace=False):
    from concourse import bass_utils
    return bass_utils.run_bass_kernel_spmd(
        _get_nc(), in_maps, list(range(NCORES)), trace=trace
    )


def _finish(res, u, v):
    partials = [r["partial"].reshape(()) for r in res.results]
    lnsum = np.sum(np.asarray(partials, dtype=np.float64))
    log_num = np.sum(u.astype(np.float64)) - np.sum(v.astype(np.float64))
    loss = np.float32(lnsum - log_num)
    return np.asarray(loss, dtype=np.float32).reshape(())


def kernel(pred, target):
    in_maps, u, v = _make_in_maps(pred, target)
    res = _run(in_maps)
    return _finish(res, u, v)


def kernel_traced(pred, target):
    in_maps, u, v = _make_in_maps(pred, target)
    res = _run(in_maps, trace=True)
    return _finish(res, u, v), res
